# revision 3
# baseline (speedup 1.0000x reference)
"""MoE routing (gate) kernel for Trainium2, 8 NeuronCores, data-parallel.

Computes, for x [65536, 4096] f32 and W [64, 4096] f32:
    logits  = x @ W.T                       # [65536, 64]
    scores  = softmax(logits, axis=-1)
    weights, indices = top_k(scores, 8)     # [65536, 8] each
    weights *= 2.5

Sharding: token dim split 8 ways (8192 tokens/core); W replicated.
Host-side prep: x is transposed to [4096, tokens] per shard so the
contraction dim (d) lands on SBUF partitions, and W is transposed to
W.T [4096, 64] so each 128-row chunk is a ready matmul stationary.

Per-core program (Tile framework), for each group of 512 tokens:
  - 32 accumulating PE matmuls: logitsT[64, 512] += WT_k.T @ xT_k
  - copy PSUM->SBUF, 4 PE transposes -> logits [128 tok, 64 exp]
  - DVE max/max_index -> top-8 values + indices (desc order, first-index
    tie-break = jax.lax.top_k order)
  - ACT exp(x - max) with accumulated row-sum -> softmax denominator
  - weights = exp(top8 - max) * 2.5 / denom
"""

import os
import sys

for _p in ("/opt/trn_rl_repo", "/root/.axon_site/_ro/trn_rl_repo"):
    if os.path.isdir(_p) and _p not in sys.path:
        sys.path.append(_p)

import numpy as np

import concourse.bass as bass
import concourse.mybir as mybir
from concourse import masks, tile
from concourse.bass_utils import run_bass_kernel_spmd
from concourse.vector_clock import ScopedClock

TOKENS = 65536
D = 4096
E = 64
TOPK = 8
ROUTE_SCALE = 2.5
N_CORES = 8
T_CORE = TOKENS // N_CORES  # 8192
T_G = 512                   # tokens per group (one PSUM bank at fp32)
N_G = T_CORE // T_G         # 16
KC = D // 128               # 32 contraction chunks

F32 = mybir.dt.float32
I32 = mybir.dt.int32
U32 = mybir.dt.uint32

# ---------------------------------------------------------------------------
# Walrus in this container rejects >1 sync-wait on control instructions; the
# stock TileContext tail drain carries one wait per live processor.  Spread
# them across sync-engine NOPs (1 each) before the drain.
_MAX_WAITS = 1


def _patched_drain_and_barrier(self, tick_clock, wait_clock):
    nc = self.nc
    probe = nc.sync.nop()
    wait_clock.add_sem_waits(probe.ins, ScopedClock({None: tick_clock.global_clock}))
    waits = list(probe.ins.sync_info.on_wait or [])
    probe.ins.sync_info.on_wait = waits[:_MAX_WAITS]
    for i in range(_MAX_WAITS, len(waits), _MAX_WAITS):
        extra = nc.sync.nop()
        if extra.ins.sync_info is None:
            extra.ins.sync_info = mybir.SyncInfo(
                on_wait=waits[i : i + _MAX_WAITS], on_update=[]
            )
        else:
            extra.ins.sync_info.on_wait = waits[i : i + _MAX_WAITS]
    nc.sync.drain()

    nc.all_engine_barrier()
    assert self.sems is not None
    popped = nc._tile_sem_poison_stack.pop()
    assert popped is self._sem_poison
    nc.clear_and_free_semaphores(list(self.sems.allocated().values()))
    nc.all_engine_barrier()


tile.TileContext._drain_and_barrier = _patched_drain_and_barrier


def _split_multi_waits(nc: bass.Bass, max_waits: int = _MAX_WAITS):
    """Walrus here caps sync waits at 1 per instruction (any engine struct).
    Hoist excess waits onto same-engine NOPs inserted just before the
    offending instruction — the sequencer satisfies them in order, so the
    semantics (AND of all waits before execute) are preserved."""
    n = 0
    for fn in nc.m.functions:
        for bb in fn.blocks:
            out = []
            changed = False
            for inst in bb.instructions:
                si = inst.sync_info
                w = list(si.on_wait) if (si and si.on_wait) else []
                if len(w) > max_waits:
                    extras = w[: len(w) - max_waits]
                    si.on_wait = w[len(w) - max_waits :]
                    for i0 in range(0, len(extras), max_waits):
                        nop = mybir.InstNoOp(
                            name=f"I-wsplit-{nc.next_id()}", ins=[], outs=[]
                        )
                        nop.engine = inst.engine
                        nop.sync_info = mybir.SyncInfo(
                            on_wait=extras[i0 : i0 + max_waits], on_update=[]
                        )
                        out.append(nop)
                        n += 1
                    changed = True
                out.append(inst)
            if changed:
                bb.instructions = out
    return n
# ---------------------------------------------------------------------------

MM_DTYPE = os.environ.get("GATE_MM_DTYPE", "f32")  # "f32" | "f32r"


def _mm_ap(ap):
    if MM_DTYPE == "f32r":
        return ap.bitcast(mybir.dt.float32r)
    return ap


def _build_program() -> bass.Bass:
    nc = bass.Bass()
    xt = nc.declare_dram_parameter("xt", [D, T_CORE], F32, isOutput=False)
    wt = nc.declare_dram_parameter("wt", [D, E], F32, isOutput=False)
    w_out = nc.declare_dram_parameter("w_out", [T_CORE, TOPK], F32, isOutput=True)
    i_out = nc.declare_dram_parameter("i_out", [T_CORE, TOPK], I32, isOutput=True)

    with tile.TileContext(nc) as tc:
        with (
            tc.tile_pool(name="const", bufs=1) as const_pool,
            tc.tile_pool(name="xin", bufs=8) as xpool,
            tc.tile_pool(name="lsb", bufs=2) as lspool,
            tc.tile_pool(name="lg", bufs=4) as lgpool,
            tc.tile_pool(name="epi", bufs=4) as epool,
            tc.tile_pool(name="outg", bufs=2) as opool,
            tc.tile_pool(name="ps_l", bufs=2, space="PSUM") as ps_l,
            tc.tile_pool(name="ps_t", bufs=4, space="PSUM") as ps_t,
        ):
            ident = const_pool.tile([128, 128], F32)
            masks.make_identity(nc, ident[:])

            # W.T staged as [128, KC, E]: partition p of chunk k = W.T row k*128+p
            wt_sb = const_pool.tile([128, KC, E], F32)
            nc.sync.dma_start(wt_sb[:], wt.rearrange("(k p) e -> p k e", p=128))

            for g in range(N_G):
                logitsT = ps_l.tile([E, T_G], F32, name="logitsT")
                for k in range(KC):
                    xsb = xpool.tile([128, T_G], F32, tag="xsb")
                    nc.sync.dma_start(
                        xsb[:],
                        xt[k * 128 : (k + 1) * 128, g * T_G : (g + 1) * T_G],
                    )
                    nc.tensor.matmul(
                        logitsT[:],
                        _mm_ap(wt_sb[:, k, :]),
                        _mm_ap(xsb[:]),
                        start=(k == 0),
                        stop=(k == KC - 1),
                    )

                ls = lspool.tile([E, T_G], F32, tag="ls")
                nc.scalar.copy(ls[:], logitsT[:])

                w_grp = opool.tile([128, T_G // 128, TOPK], F32, tag="wg")
                i_grp = opool.tile([128, T_G // 128, TOPK], I32, tag="ig")

                for j in range(T_G // 128):
                    lt_ps = ps_t.tile([128, E], F32, name="lt_ps")
                    nc.tensor.transpose(
                        lt_ps[:], ls[:, j * 128 : (j + 1) * 128], ident[:E, :E]
                    )
                    lg = lgpool.tile([128, E], F32, tag="lg")
                    nc.vector.tensor_copy(lg[:], lt_ps[:])

                    mx8 = epool.tile([128, TOPK], F32, tag="mx8")
                    nc.vector.max(mx8[:], lg[:])
                    nc.vector.max_index(
                        i_grp[:, j, :].bitcast(U32), mx8[:], lg[:]
                    )

                    negmax = epool.tile([128, 1], F32, tag="negmax")
                    nc.scalar.mul(negmax[:], mx8[:, 0:1], -1.0)

                    expall = epool.tile([128, E], F32, tag="expall")
                    denom = epool.tile([128, 1], F32, tag="denom")
                    nc.scalar.activation(
                        expall[:],
                        lg[:],
                        mybir.ActivationFunctionType.Exp,
                        bias=negmax[:],
                        accum_out=denom[:],
                    )
                    exp8 = epool.tile([128, TOPK], F32, tag="exp8")
                    nc.scalar.activation(
                        exp8[:],
                        mx8[:],
                        mybir.ActivationFunctionType.Exp,
                        bias=negmax[:],
                    )
                    r25 = epool.tile([128, 1], F32, tag="r25")
                    nc.vector.reciprocal(r25[:], denom[:])
                    nc.scalar.mul(r25[:], r25[:], ROUTE_SCALE)
                    nc.vector.tensor_scalar_mul(w_grp[:, j, :], exp8[:], r25[:])

                nc.sync.dma_start(
                    w_out[g * T_G : (g + 1) * T_G, :].rearrange(
                        "(j p) e -> p j e", p=128
                    ),
                    w_grp[:],
                )
                nc.sync.dma_start(
                    i_out[g * T_G : (g + 1) * T_G, :].rearrange(
                        "(j p) e -> p j e", p=128
                    ),
                    i_grp[:],
                )

    _split_multi_waits(nc)
    return nc


_NC = None


def _get_program() -> bass.Bass:
    global _NC
    if _NC is None:
        _NC = _build_program()
    return _NC


def _run(x: np.ndarray, W: np.ndarray, **kwargs):
    x = np.asarray(x, dtype=np.float32)
    W = np.asarray(W, dtype=np.float32)
    assert x.shape == (TOKENS, D), x.shape
    assert W.shape == (E, D), W.shape

    wt_host = np.ascontiguousarray(W.T)  # [D, E]
    in_maps = []
    for c in range(N_CORES):
        shard = np.ascontiguousarray(x[c * T_CORE : (c + 1) * T_CORE, :].T)
        in_maps.append({"xt": shard, "wt": wt_host})

    nc = _get_program()
    res = run_bass_kernel_spmd(nc, in_maps, core_ids=list(range(N_CORES)), **kwargs)

    weights = np.concatenate([res.results[c]["w_out"] for c in range(N_CORES)], axis=0)
    indices = np.concatenate([res.results[c]["i_out"] for c in range(N_CORES)], axis=0)
    return weights.astype(np.float32), indices.astype(np.int32), res


def kernel(x: np.ndarray, W: np.ndarray):
    weights, indices, _ = _run(x, W)
    return weights, indices
